# revision 8
# baseline (speedup 1.0000x reference)
"""Trainium2 Bass kernel for nn_MultiHeadAttention_59227599012491.

Reference computation (per batch b):
    xf = x[b].reshape(S, 256)
    q  = softplus(xf @ Wq.T + bq);  k = softplus(xf @ Wk.T + bk)
    v  = xf @ Wv.T + bv
    weight = q @ k.T            (no softmax!)
    result = weight @ v
    out    = result @ Wo.T + bo

Because there is no softmax, attention is associative:
    result = (q @ k.T) @ v = q @ (k.T @ v) = q @ G,   G: [256, 256]
    out    = q @ (G @ Wo.T) + bo = q @ M + bo
so the S x S score matrix never needs to be materialized. Per-core work
drops to a handful of [*, 256] x [256, 256] matmuls; the kernel is
memory-bound on streaming x in and out once.

Sharding: B=4 batches x 2 query-halves -> 8 cores, no collectives.
Each core computes k/v/G/M for its whole batch (cheap, duplicated
within a pair) and the output rows for its half of the queries.

Layouts (PE computes out = lhsT.T @ rhs, contracting partition dim):
    xbT  [256, 4096]  x[b] transposed on host (queries first SQ cols)
    qT   [256, 2048]  lhsT = WqT tile, rhs = xbT     (softplus via ACT,
                      bias per-partition, fused into the Exp pass)
    kv   [4096, 512]  k and v fused: rhs = [WkT | WvT], one stationary
                      xbT tile per row tile serves both. +[bk|bv] via a
                      single DVE add; softplus on the k half in-place
                      (ACT Exp then Ln(1+t), batched over tile pairs)
    GT   [256, 256]   GT[d,e] = sum_s v[s,d] k[s,e]: lhsT = v t, rhs = k t
    M    [256, 256]   M[e,do] = sum_d GT[d,e] WoT[d,do]: lhsT = GT, rhs = WoT
    out  [2048, 256]  lhsT = qT tile, rhs = M        (bias via DVE add)

float32r streams at 1 cycle/row on the PE (vs 4 for exact float32);
hardware requires both matmul operands f32r, even free-dim counts, and
8B-aligned PSUM destinations, so the tiny K=1 bias/broadcast matmuls
stay plain fp32.

The activation-table pass is steered to `natural_log_exp_and_others`
(the only set holding Exp AND Ln) so the ACT engine loads its PWP table
once instead of reloading per activation (24 loads ~= 30us saved).
"""

import numpy as np

S = 4096
SQ = 2048  # query rows per core
D = 256
P = 128
IT = D // P  # 2 input-dim tiles
DT = D // P  # 2 d-model tiles
NS = S // P  # 32 sequence tiles
BLK = 512  # free-dim block for qT
N_CORES = 8

MM_DTYPE_NAME = "float32r"

_CACHE = {}


def _patched_act_tables(orig_fn):
    def patched(arch):
        tabs = orig_fn(arch)
        return {
            name: (s if name == "natural_log_exp_and_others" else set())
            for name, s in tabs.items()
        }

    return patched


def _build_nc():
    import concourse.bacc as bacc
    import concourse.mybir as mybir
    import concourse.tile as tile

    FP = mybir.dt.float32
    FR = getattr(mybir.dt, MM_DTYPE_NAME)
    AF = mybir.ActivationFunctionType
    ADD = mybir.AluOpType.add

    nc = bacc.Bacc("TRN2", target_bir_lowering=False, debug=False, num_devices=1)

    xbT_d = nc.declare_dram_parameter("xbT", [D, S], FR, isOutput=False)
    wqT_d = nc.declare_dram_parameter("wqT", [D, D], FR, isOutput=False)
    wkvT_d = nc.declare_dram_parameter("wkvT", [D, 2 * D], FR, isOutput=False)
    woT_d = nc.declare_dram_parameter("woT", [D, D], FR, isOutput=False)
    bq_d = nc.declare_dram_parameter("bq", [1, D], FP, isOutput=False)
    bkv_d = nc.declare_dram_parameter("bkv", [1, 2 * D], FP, isOutput=False)
    bo2_d = nc.declare_dram_parameter("bo2", [1, 2 * D], FP, isOutput=False)
    out_d = nc.declare_dram_parameter("out", [SQ, D], FP, isOutput=True)

    def mm(psum, lhsT, rhs, start, stop):
        nc.tensor.matmul(psum, lhsT, rhs, start=start, stop=stop)

    with tile.TileContext(nc) as tc:
        with (
            tc.tile_pool(name="w", bufs=1) as wpool,
            tc.tile_pool(name="big", bufs=1) as big,
            tc.tile_pool(name="tmp", bufs=4) as tpool,
            tc.tile_pool(name="ob", bufs=4) as opool,
            tc.tile_pool(name="psQ", bufs=3, space="PSUM") as psQ,
            tc.tile_pool(name="psKV", bufs=3, space="PSUM") as psKV,
            tc.tile_pool(name="psG", bufs=2, space="PSUM") as psG,
        ):
            # --- load weights / biases; wkv + leading xbT chunks first
            # so the kv matmuls can start as soon as possible ---
            wq_sb = wpool.tile([P, IT, D], FR, tag="wq")
            wo_sb = wpool.tile([P, IT, D], FR, tag="wo")
            wkv_sb = wpool.tile([P, IT, 2 * D], FR, tag="wkv")
            xbT_sb = big.tile([P, IT, S], FR, tag="xbT")
            bq_sb = wpool.tile([1, D], FP, tag="bq")
            bkv_sb = wpool.tile([1, 2 * D], FP, tag="bkv")
            bo2_sb = wpool.tile([1, 2 * D], FP, tag="bo2")
            nc.sync.dma_start(bq_sb[:, :], bq_d.ap())
            nc.sync.dma_start(bkv_sb[:, :], bkv_d.ap())
            nc.sync.dma_start(bo2_sb[:, :], bo2_d.ap())
            for it in range(IT):
                rows = slice(it * P, (it + 1) * P)
                nc.sync.dma_start(wkv_sb[:, it, :], wkvT_d.ap()[rows, :])
                nc.sync.dma_start(wq_sb[:, it, :], wqT_d.ap()[rows, :])
                nc.sync.dma_start(wo_sb[:, it, :], woT_d.ap()[rows, :])
            ones = wpool.tile([1, P], FP, tag="ones")
            nc.gpsimd.memset(ones[:, :], 1.0)
            for sc in range(4):
                cols = slice(sc * 1024, (sc + 1) * 1024)
                for it in range(IT):
                    nc.sync.dma_start(
                        xbT_sb[:, it, cols],
                        xbT_d.ap()[it * P : (it + 1) * P, cols],
                    )

            # bias helper tiles (built via tiny fp32 K=1 matmuls):
            #   bqT [128, DT] per-partition column for qT's fused ACT bias
            #   bkv_bc / bo2_bc [128, 512] broadcast rows for DVE adds
            bqT = wpool.tile([P, DT], FP, tag="bqT")
            for dt in range(DT):
                ps = psKV.tile([P, 2 * D], FP, tag="psKV")
                nc.tensor.matmul(
                    ps[:, 0:1], bq_sb[0:1, dt * P : (dt + 1) * P], ones[0:1, 0:1],
                    start=True, stop=True,
                )
                nc.vector.tensor_copy(bqT[:, dt : dt + 1], ps[:, 0:1])
            b_bc = {}
            for nm, bsrc in (("bkv", bkv_sb), ("bo2", bo2_sb)):
                ps = psKV.tile([P, 2 * D], FP, tag="psKV")
                nc.tensor.matmul(ps[:, :], ones[0:1, :], bsrc[0:1, :], start=True, stop=True)
                bc = wpool.tile([P, 2 * D], FP, tag=f"bc_{nm}")
                nc.vector.tensor_copy(bc[:, :], ps[:, :])
                b_bc[nm] = bc

            # kv planes: kv_sb[:, 0, t, :] = k (post-softplus),
            #            kv_sb[:, 1, t, :] = v
            kv_sb = big.tile([P, 2, NS, D], FR, tag="kv")
            qT_sb = big.tile([P, DT, SQ], FR, tag="qT")
            GT_sb = wpool.tile([P, DT, D], FR, tag="GT")
            M_sb = wpool.tile([P, DT, D], FR, tag="M")

            # --- kv = x [WkT | WvT] + [bk | bv] ---
            for t in range(NS):
                ts = slice(t * P, (t + 1) * P)
                ps = psKV.tile([P, 2 * D], FP, tag="psKV")
                for it in range(IT):
                    mm(ps[:, :], xbT_sb[:, it, ts], wkv_sb[:, it, :], it == 0, it == IT - 1)
                nc.vector.tensor_tensor(
                    kv_sb[:, :, t, :], ps[:, :].rearrange("p (j d) -> p j d", j=2),
                    b_bc["bkv"][:, :].rearrange("p (j d) -> p j d", j=2), op=ADD,
                )
                # softplus on contiguous k-plane runs of 4 row tiles
                if t % 4 == 3:
                    tt = slice(t - 3, t + 1)
                    tmp = tpool.tile([P, 4, D], FP, tag="tmpk")
                    nc.scalar.activation(tmp[:, :, :], kv_sb[:, 0, tt, :].bitcast(FP), AF.Exp)
                    nc.scalar.activation(kv_sb[:, 0, tt, :], tmp[:, :, :], AF.Ln, bias=1.0)

            # --- qT = softplus(Wq x^T + bq), transposed layout [e, sq] ---
            for dt in range(DT):
                ds = slice(dt * P, (dt + 1) * P)
                for blk in range(SQ // BLK):
                    ss = slice(blk * BLK, (blk + 1) * BLK)
                    ps = psQ.tile([P, BLK], FP, tag="psQ")
                    for it in range(IT):
                        mm(ps[:, :], wq_sb[:, it, ds], xbT_sb[:, it, ss], it == 0, it == IT - 1)
                    tmp = tpool.tile([P, BLK], FP, tag="tmpq")
                    nc.scalar.activation(tmp[:, :], ps[:, :], AF.Exp, bias=bqT[:, dt : dt + 1])
                    nc.scalar.activation(qT_sb[:, dt, ss], tmp[:, :], AF.Ln, bias=1.0)

            # --- GT[d, e] = sum_s v[s, d] k[s, e] ---
            for dt in range(DT):
                vs = slice(dt * P, (dt + 1) * P)
                ps = psG.tile([P, D], FP, tag="psG")
                for t in range(NS):
                    mm(ps[:, :], kv_sb[:, 1, t, vs], kv_sb[:, 0, t, :], t == 0, t == NS - 1)
                nc.vector.tensor_copy(GT_sb[:, dt, :], ps[:, :])

            # --- M[e, do] = sum_d GT[d, e] WoT[d, do] ---
            for et in range(DT):
                es = slice(et * P, (et + 1) * P)
                ps = psG.tile([P, D], FP, tag="psG")
                for dt in range(DT):
                    mm(ps[:, :], GT_sb[:, dt, es], wo_sb[:, dt, :], dt == 0, dt == DT - 1)
                nc.vector.tensor_copy(M_sb[:, et, :], ps[:, :])

            # --- out[sq, do] = sum_e q[sq, e] M[e, do] + bo, pairs ---
            for pr in range(SQ // (2 * P)):
                ps = psQ.tile([P, 2, D], FP, tag="psQ")
                for j in range(2):
                    ss = slice((2 * pr + j) * P, (2 * pr + j + 1) * P)
                    for et in range(DT):
                        mm(ps[:, j, :], qT_sb[:, et, ss], M_sb[:, et, :], et == 0, et == DT - 1)
                ob = opool.tile([P, 2, D], FP, tag="ob")
                nc.vector.tensor_tensor(
                    ob[:, :, :], ps[:, :, :],
                    b_bc["bo2"][:, :].rearrange("p (j d) -> p j d", j=2), op=ADD,
                )
                nc.sync.dma_start(
                    out_d.ap()[2 * pr * P : (2 * pr + 2) * P, :].rearrange(
                        "(j p) d -> p j d", p=P
                    ),
                    ob[:, :, :],
                )

    # Steer the activation-table pass: only natural_log_exp_and_others
    # (set 6) contains both Exp and Ln, so one PWP table load suffices.
    import concourse.hw_specs as hw_specs

    orig = bacc.get_activation_tables
    bacc.get_activation_tables = _patched_act_tables(hw_specs.get_activation_tables)
    try:
        nc.compile()
    finally:
        bacc.get_activation_tables = orig
    return nc


def _get_nc():
    nc = _CACHE.get("nc")
    if nc is None:
        nc = _build_nc()
        _CACHE["nc"] = nc
    return nc


def make_in_maps(x, Wq, bq, Wk, bk, Wv, bv, Wo, bo):
    B = x.shape[0]
    xf = np.asarray(x, dtype=np.float32).reshape(B, S, D)
    xfT = np.ascontiguousarray(xf.transpose(0, 2, 1))  # [B, 256, 4096]
    shared = {
        "wqT": np.ascontiguousarray(np.asarray(Wq, np.float32).T),
        "wkvT": np.ascontiguousarray(
            np.hstack([np.asarray(Wk, np.float32).T, np.asarray(Wv, np.float32).T])
        ),
        "woT": np.ascontiguousarray(np.asarray(Wo, np.float32).T),
        "bq": np.asarray(bq, np.float32).reshape(1, D),
        "bkv": np.concatenate(
            [np.asarray(bk, np.float32), np.asarray(bv, np.float32)]
        ).reshape(1, 2 * D),
        "bo2": np.tile(np.asarray(bo, np.float32), 2).reshape(1, 2 * D),
    }
    in_maps = []
    for c in range(N_CORES):
        b, h = divmod(c, 2)
        xT = xfT[b]
        if h == 1:
            xT = np.concatenate([xT[:, SQ:], xT[:, :SQ]], axis=1)
        in_maps.append({"xbT": np.ascontiguousarray(xT), **shared})
    return in_maps


def assemble_out(results, x_shape):
    B, S_, H, W = x_shape
    out = np.empty((B, S_, D), np.float32)
    for c in range(N_CORES):
        b, h = divmod(c, 2)
        out[b, h * SQ : (h + 1) * SQ] = results[c]["out"]
    return out.reshape(B, S_, H, W)


def kernel(x, Wq, bq, Wk, bk, Wv, bv, Wo, bo, _trace=False):
    from concourse.bass_utils import run_bass_kernel_spmd

    nc = _get_nc()
    in_maps = make_in_maps(x, Wq, bq, Wk, bk, Wv, bv, Wo, bo)
    res = run_bass_kernel_spmd(nc, in_maps, list(range(N_CORES)), trace=_trace)
    out = assemble_out(res.results, x.shape)
    if _trace:
        _CACHE["last_result"] = res
    return out


# revision 10
# speedup vs baseline: 1.4449x; 1.4449x over previous
"""Trainium2 Bass kernel for nn_MultiHeadAttention_59227599012491.

Reference computation (per batch b):
    xf = x[b].reshape(S, 256)
    q  = softplus(xf @ Wq.T + bq);  k = softplus(xf @ Wk.T + bk)
    v  = xf @ Wv.T + bv
    weight = q @ k.T            (no softmax!)
    result = weight @ v
    out    = result @ Wo.T + bo

Because there is no softmax, attention is associative:
    result = (q @ k.T) @ v = q @ (k.T @ v) = q @ G,   G: [256, 256]
    out    = q @ (G @ Wo.T) + bo = q @ M + bo
so the S x S score matrix never needs to be materialized. Per-core work
drops to a handful of [*, 256] x [256, 256] matmuls; the kernel is
memory-bound on streaming x in and out once.

Sharding: B=4 batches x 2 query-halves -> 8 cores, no collectives.
Each core computes k/v/G/M for its whole batch (cheap, duplicated
within a pair) and the output rows for its half of the queries.

Layouts (PE computes out = lhsT.T @ rhs, contracting partition dim):
    xbT  [256, 4096]  x[b] transposed on host (queries first SQ cols)
    qT   [256, 2048]  lhsT = WqT tile, rhs = xbT     (softplus via ACT,
                      bias per-partition, fused into the Exp pass)
    kv   [4096, 512]  k and v fused: rhs = [WkT | WvT], one stationary
                      xbT tile per row tile serves both. +[bk|bv] via a
                      single DVE add; softplus on the k half in-place
                      (ACT Exp then Ln(1+t), batched over tile pairs)
    GT   [256, 256]   GT[d,e] = sum_s v[s,d] k[s,e]: lhsT = v t, rhs = k t
    M    [256, 256]   M[e,do] = sum_d GT[d,e] WoT[d,do]: lhsT = GT, rhs = WoT
    out  [2048, 256]  lhsT = qT tile, rhs = M        (bias via DVE add)

float32r streams at 1 cycle/row on the PE (vs 4 for exact float32);
hardware requires both matmul operands f32r, even free-dim counts, and
8B-aligned PSUM destinations, so the tiny K=1 bias/broadcast matmuls
stay plain fp32.

The activation-table pass is steered to `natural_log_exp_and_others`
(the only set holding Exp AND Ln) so the ACT engine loads its PWP table
once instead of reloading per activation (24 loads ~= 30us saved).
"""

import numpy as np

S = 4096
SQ = 2048  # query rows per core
D = 256
P = 128
IT = D // P  # 2 input-dim tiles
DT = D // P  # 2 d-model tiles
NS = S // P  # 32 sequence tiles
BLK = 512  # free-dim block for qT
N_CORES = 8

MM_DTYPE_NAME = "float16"

_CACHE = {}


def _patched_act_tables(orig_fn):
    def patched(arch):
        tabs = orig_fn(arch)
        return {
            name: (s if name == "natural_log_exp_and_others" else set())
            for name, s in tabs.items()
        }

    return patched


def _build_nc():
    import concourse.bacc as bacc
    import concourse.mybir as mybir
    import concourse.tile as tile

    FP = mybir.dt.float32
    FR = getattr(mybir.dt, MM_DTYPE_NAME)
    AF = mybir.ActivationFunctionType
    ADD = mybir.AluOpType.add

    nc = bacc.Bacc("TRN2", target_bir_lowering=False, debug=False, num_devices=1)

    xbT_d = nc.declare_dram_parameter("xbT", [D, S], FR, isOutput=False)
    wqT_d = nc.declare_dram_parameter("wqT", [D, D], FR, isOutput=False)
    wkvT_d = nc.declare_dram_parameter("wkvT", [D, 2 * D], FR, isOutput=False)
    woT_d = nc.declare_dram_parameter("woT", [D, D], FR, isOutput=False)
    bq_d = nc.declare_dram_parameter("bq", [1, D], FP, isOutput=False)
    bkv_d = nc.declare_dram_parameter("bkv", [1, 2 * D], FP, isOutput=False)
    bo2_d = nc.declare_dram_parameter("bo2", [1, 2 * D], FP, isOutput=False)
    out_d = nc.declare_dram_parameter("out", [SQ, D], FP, isOutput=True)

    def mm(psum, lhsT, rhs, start, stop):
        nc.tensor.matmul(psum, lhsT, rhs, start=start, stop=stop)

    with tile.TileContext(nc) as tc:
        with (
            tc.tile_pool(name="w", bufs=1) as wpool,
            tc.tile_pool(name="big", bufs=1) as big,
            tc.tile_pool(name="tmp", bufs=4) as tpool,
            tc.tile_pool(name="ob", bufs=4) as opool,
            tc.tile_pool(name="psQ", bufs=3, space="PSUM") as psQ,
            tc.tile_pool(name="psKV", bufs=3, space="PSUM") as psKV,
            tc.tile_pool(name="psG", bufs=2, space="PSUM") as psG,
        ):
            # --- load weights / biases; wkv + leading xbT chunks first
            # so the kv matmuls can start as soon as possible ---
            wq_sb = wpool.tile([P, IT, D], FR, tag="wq")
            wo_sb = wpool.tile([P, IT, D], FR, tag="wo")
            wkv_sb = wpool.tile([P, IT, 2 * D], FR, tag="wkv")
            xbT_sb = big.tile([P, IT, S], FR, tag="xbT")
            bq_sb = wpool.tile([1, D], FP, tag="bq")
            bkv_sb = wpool.tile([1, 2 * D], FP, tag="bkv")
            bo2_sb = wpool.tile([1, 2 * D], FP, tag="bo2")
            nc.sync.dma_start(bq_sb[:, :], bq_d.ap())
            nc.sync.dma_start(bkv_sb[:, :], bkv_d.ap())
            nc.sync.dma_start(bo2_sb[:, :], bo2_d.ap())
            for it in range(IT):
                rows = slice(it * P, (it + 1) * P)
                nc.sync.dma_start(wkv_sb[:, it, :], wkvT_d.ap()[rows, :])
                nc.sync.dma_start(wq_sb[:, it, :], wqT_d.ap()[rows, :])
                nc.sync.dma_start(wo_sb[:, it, :], woT_d.ap()[rows, :])
            ones = wpool.tile([1, P], FP, tag="ones")
            nc.gpsimd.memset(ones[:, :], 1.0)
            for sc in range(4):
                cols = slice(sc * 1024, (sc + 1) * 1024)
                for it in range(IT):
                    nc.sync.dma_start(
                        xbT_sb[:, it, cols],
                        xbT_d.ap()[it * P : (it + 1) * P, cols],
                    )

            # bias helper tiles (built via tiny fp32 K=1 matmuls):
            #   bqT [128, DT] per-partition column for qT's fused ACT bias
            #   bkv_bc / bo2_bc [128, 512] broadcast rows for DVE adds
            bqT = wpool.tile([P, DT], FP, tag="bqT")
            for dt in range(DT):
                ps = psKV.tile([P, 2 * D], FP, tag="psKV")
                nc.tensor.matmul(
                    ps[:, 0:1], bq_sb[0:1, dt * P : (dt + 1) * P], ones[0:1, 0:1],
                    start=True, stop=True,
                )
                nc.vector.tensor_copy(bqT[:, dt : dt + 1], ps[:, 0:1])
            b_bc = {}
            for nm, bsrc in (("bkv", bkv_sb), ("bo2", bo2_sb)):
                ps = psKV.tile([P, 2 * D], FP, tag="psKV")
                nc.tensor.matmul(ps[:, :], ones[0:1, :], bsrc[0:1, :], start=True, stop=True)
                bc = wpool.tile([P, 2 * D], FP, tag=f"bc_{nm}")
                nc.vector.tensor_copy(bc[:, :], ps[:, :])
                b_bc[nm] = bc

            # kv planes: kv_sb[:, 0, t, :] = k (post-softplus),
            #            kv_sb[:, 1, t, :] = v
            kv_sb = big.tile([P, 2, NS, D], FR, tag="kv")
            qT_sb = big.tile([P, DT, SQ], FR, tag="qT")
            GT_sb = wpool.tile([P, DT, D], FR, tag="GT")
            M_sb = wpool.tile([P, DT, D], FR, tag="M")

            # --- kv = x [WkT | WvT] + [bk | bv] ---
            for t in range(NS):
                ts = slice(t * P, (t + 1) * P)
                ps = psKV.tile([P, 2 * D], FP, tag="psKV")
                for it in range(IT):
                    mm(ps[:, :], xbT_sb[:, it, ts], wkv_sb[:, it, :], it == 0, it == IT - 1)
                nc.vector.tensor_tensor(
                    kv_sb[:, :, t, :], ps[:, :].rearrange("p (j d) -> p j d", j=2),
                    b_bc["bkv"][:, :].rearrange("p (j d) -> p j d", j=2), op=ADD,
                )
                # softplus on contiguous k-plane runs of 4 row tiles
                if t % 4 == 3:
                    tt = slice(t - 3, t + 1)
                    tmp = tpool.tile([P, 4, D], FP, tag="tmpk")
                    nc.scalar.activation(tmp[:, :, :], kv_sb[:, 0, tt, :], AF.Exp)
                    nc.scalar.activation(kv_sb[:, 0, tt, :], tmp[:, :, :], AF.Ln, bias=1.0)

            # --- qT = softplus(Wq x^T + bq), transposed layout [e, sq] ---
            for dt in range(DT):
                ds = slice(dt * P, (dt + 1) * P)
                for blk in range(SQ // BLK):
                    ss = slice(blk * BLK, (blk + 1) * BLK)
                    ps = psQ.tile([P, BLK], FP, tag="psQ")
                    for it in range(IT):
                        mm(ps[:, :], wq_sb[:, it, ds], xbT_sb[:, it, ss], it == 0, it == IT - 1)
                    tmp = tpool.tile([P, BLK], FP, tag="tmpq")
                    nc.scalar.activation(tmp[:, :], ps[:, :], AF.Exp, bias=bqT[:, dt : dt + 1])
                    nc.scalar.activation(qT_sb[:, dt, ss], tmp[:, :], AF.Ln, bias=1.0)

            # --- GT[d, e] = sum_s v[s, d] k[s, e] ---
            for dt in range(DT):
                vs = slice(dt * P, (dt + 1) * P)
                ps = psG.tile([P, D], FP, tag="psG")
                for t in range(NS):
                    mm(ps[:, :], kv_sb[:, 1, t, vs], kv_sb[:, 0, t, :], t == 0, t == NS - 1)
                nc.vector.tensor_copy(GT_sb[:, dt, :], ps[:, :])

            # --- M[e, do] = sum_d GT[d, e] WoT[d, do] ---
            for et in range(DT):
                es = slice(et * P, (et + 1) * P)
                ps = psG.tile([P, D], FP, tag="psG")
                for dt in range(DT):
                    mm(ps[:, :], GT_sb[:, dt, es], wo_sb[:, dt, :], dt == 0, dt == DT - 1)
                nc.vector.tensor_copy(M_sb[:, et, :], ps[:, :])

            # --- out[sq, do] = sum_e q[sq, e] M[e, do] + bo, pairs ---
            for pr in range(SQ // (2 * P)):
                ps = psQ.tile([P, 2, D], FP, tag="psQ")
                for j in range(2):
                    ss = slice((2 * pr + j) * P, (2 * pr + j + 1) * P)
                    for et in range(DT):
                        mm(ps[:, j, :], qT_sb[:, et, ss], M_sb[:, et, :], et == 0, et == DT - 1)
                ob = opool.tile([P, 2, D], FP, tag="ob")
                nc.vector.tensor_tensor(
                    ob[:, :, :], ps[:, :, :],
                    b_bc["bo2"][:, :].rearrange("p (j d) -> p j d", j=2), op=ADD,
                )
                nc.sync.dma_start(
                    out_d.ap()[2 * pr * P : (2 * pr + 2) * P, :].rearrange(
                        "(j p) d -> p j d", p=P
                    ),
                    ob[:, :, :],
                )

    # Steer the activation-table pass: only natural_log_exp_and_others
    # (set 6) contains both Exp and Ln, so one PWP table load suffices.
    import concourse.hw_specs as hw_specs

    orig = bacc.get_activation_tables
    bacc.get_activation_tables = _patched_act_tables(hw_specs.get_activation_tables)
    try:
        nc.compile()
    finally:
        bacc.get_activation_tables = orig
    return nc


def _get_nc():
    nc = _CACHE.get("nc")
    if nc is None:
        nc = _build_nc()
        _CACHE["nc"] = nc
    return nc


def make_in_maps(x, Wq, bq, Wk, bk, Wv, bv, Wo, bo):
    B = x.shape[0]
    mmnp = np.float16 if MM_DTYPE_NAME == "float16" else np.float32
    xf = np.asarray(x, dtype=np.float32).reshape(B, S, D)
    xfT = np.ascontiguousarray(xf.transpose(0, 2, 1).astype(mmnp))  # [B, 256, 4096]
    shared = {
        "wqT": np.ascontiguousarray(np.asarray(Wq, mmnp).T),
        "wkvT": np.ascontiguousarray(
            np.hstack([np.asarray(Wk, mmnp).T, np.asarray(Wv, mmnp).T])
        ),
        "woT": np.ascontiguousarray(np.asarray(Wo, mmnp).T),
        "bq": np.asarray(bq, np.float32).reshape(1, D),
        "bkv": np.concatenate(
            [np.asarray(bk, np.float32), np.asarray(bv, np.float32)]
        ).reshape(1, 2 * D),
        "bo2": np.tile(np.asarray(bo, np.float32), 2).reshape(1, 2 * D),
    }
    in_maps = []
    for c in range(N_CORES):
        b, h = divmod(c, 2)
        xT = xfT[b]
        if h == 1:
            xT = np.concatenate([xT[:, SQ:], xT[:, :SQ]], axis=1)
        in_maps.append({"xbT": np.ascontiguousarray(xT), **shared})
    return in_maps


def assemble_out(results, x_shape):
    B, S_, H, W = x_shape
    out = np.empty((B, S_, D), np.float32)
    for c in range(N_CORES):
        b, h = divmod(c, 2)
        out[b, h * SQ : (h + 1) * SQ] = results[c]["out"]
    return out.reshape(B, S_, H, W)


def kernel(x, Wq, bq, Wk, bk, Wv, bv, Wo, bo, _trace=False):
    from concourse.bass_utils import run_bass_kernel_spmd

    nc = _get_nc()
    in_maps = make_in_maps(x, Wq, bq, Wk, bk, Wv, bv, Wo, bo)
    res = run_bass_kernel_spmd(nc, in_maps, list(range(N_CORES)), trace=_trace)
    out = assemble_out(res.results, x.shape)
    if _trace:
        _CACHE["last_result"] = res
    return out


# revision 12
# speedup vs baseline: 1.5641x; 1.0825x over previous
"""Trainium2 Bass kernel for nn_MultiHeadAttention_59227599012491.

Reference computation (per batch b):
    xf = x[b].reshape(S, 256)
    q  = softplus(xf @ Wq.T + bq);  k = softplus(xf @ Wk.T + bk)
    v  = xf @ Wv.T + bv
    weight = q @ k.T            (no softmax!)
    result = weight @ v
    out    = result @ Wo.T + bo

Because there is no softmax, attention is associative:
    result = (q @ k.T) @ v = q @ (k.T @ v) = q @ G,   G: [256, 256]
    out    = q @ (G @ Wo.T) + bo = q @ M + bo
so the S x S score matrix never needs to be materialized. Per-core work
drops to a handful of [*, 256] x [256, 256] matmuls; the kernel is
memory-bound on streaming x in and out once.

Sharding: B=4 batches x 2 query-halves -> 8 cores, no collectives.
Each core computes k/v/G/M for its whole batch (cheap, duplicated
within a pair) and the output rows for its half of the queries.

Layouts (PE computes out = lhsT.T @ rhs, contracting partition dim):
    xbT  [256, 4096]  x[b] transposed on host (queries first SQ cols)
    qT   [256, 2048]  lhsT = WqT tile, rhs = xbT     (softplus via ACT,
                      bias per-partition, fused into the Exp pass)
    kv   [4096, 512]  k and v fused: rhs = [WkT | WvT], one stationary
                      xbT tile per row tile serves both. +[bk|bv] via a
                      single DVE add; softplus on the k half in-place
                      (ACT Exp then Ln(1+t), batched over tile pairs)
    GT   [256, 256]   GT[d,e] = sum_s v[s,d] k[s,e]: lhsT = v t, rhs = k t
    M    [256, 256]   M[e,do] = sum_d GT[d,e] WoT[d,do]: lhsT = GT, rhs = WoT
    out  [2048, 256]  lhsT = qT tile, rhs = M        (bias via DVE add)

float32r streams at 1 cycle/row on the PE (vs 4 for exact float32);
hardware requires both matmul operands f32r, even free-dim counts, and
8B-aligned PSUM destinations, so the tiny K=1 bias/broadcast matmuls
stay plain fp32.

The activation-table pass is steered to `natural_log_exp_and_others`
(the only set holding Exp AND Ln) so the ACT engine loads its PWP table
once instead of reloading per activation (24 loads ~= 30us saved).
"""

import numpy as np

S = 4096
SQ = 2048  # query rows per core
D = 256
P = 128
IT = D // P  # 2 input-dim tiles
DT = D // P  # 2 d-model tiles
NS = S // P  # 32 sequence tiles
BLK = 512  # free-dim block for qT
N_CORES = 8

MM_DTYPE_NAME = "float16"

_CACHE = {}


def _patched_act_tables(orig_fn):
    def patched(arch):
        tabs = orig_fn(arch)
        return {
            name: (s if name == "natural_log_exp_and_others" else set())
            for name, s in tabs.items()
        }

    return patched


def _build_nc():
    import concourse.bacc as bacc
    import concourse.mybir as mybir
    import concourse.tile as tile

    FP = mybir.dt.float32
    FR = getattr(mybir.dt, MM_DTYPE_NAME)
    AF = mybir.ActivationFunctionType
    ADD = mybir.AluOpType.add

    nc = bacc.Bacc("TRN2", target_bir_lowering=False, debug=False, num_devices=1)

    xbT_d = nc.declare_dram_parameter("xbT", [D, S], FR, isOutput=False)
    wqT_d = nc.declare_dram_parameter("wqT", [D, D], FR, isOutput=False)
    wkvT_d = nc.declare_dram_parameter("wkvT", [D, 2 * D], FR, isOutput=False)
    woT_d = nc.declare_dram_parameter("woT", [D, D], FR, isOutput=False)
    bq_d = nc.declare_dram_parameter("bq", [1, D], FP, isOutput=False)
    bkv_d = nc.declare_dram_parameter("bkv", [1, 2 * D], FP, isOutput=False)
    bo2_d = nc.declare_dram_parameter("bo2", [1, 2 * D], FP, isOutput=False)
    out_d = nc.declare_dram_parameter("out", [SQ, D], FP, isOutput=True)

    def mm(psum, lhsT, rhs, start, stop):
        nc.tensor.matmul(psum, lhsT, rhs, start=start, stop=stop)

    with tile.TileContext(nc) as tc:
        with (
            tc.tile_pool(name="w", bufs=1) as wpool,
            tc.tile_pool(name="big", bufs=1) as big,
            tc.tile_pool(name="tmp", bufs=4) as tpool,
            tc.tile_pool(name="ob", bufs=4) as opool,
            tc.tile_pool(name="psQ", bufs=3, space="PSUM") as psQ,
            tc.tile_pool(name="psKV", bufs=3, space="PSUM") as psKV,
            tc.tile_pool(name="psG", bufs=2, space="PSUM") as psG,
        ):
            # --- loads; ordered so the kv pipeline starts ASAP:
            # wkv + first xbT chunk first, bias tiles via DMA tricks
            # (partition-strided for bqT, DRAM-broadcast for the bias
            # rows) so the PE never touches bias setup ---
            wq_sb = wpool.tile([P, IT, D], FR, tag="wq")
            wo_sb = wpool.tile([P, IT, D], FR, tag="wo")
            wkv_sb = wpool.tile([P, IT, 2 * D], FR, tag="wkv")
            xbT_sb = big.tile([P, IT, S], FR, tag="xbT")
            bqT = wpool.tile([P, DT], FP, tag="bqT")
            bc_bkv = wpool.tile([P, 2 * D], FP, tag="bc_bkv")
            bc_bo2 = wpool.tile([P, 2 * D], FP, tag="bc_bo2")
            b_bc = {"bkv": bc_bkv, "bo2": bc_bo2}
            for it in range(IT):
                nc.sync.dma_start(wkv_sb[:, it, :], wkvT_d.ap()[it * P : (it + 1) * P, :])
            for it in range(IT):
                nc.sync.dma_start(
                    xbT_sb[:, it, 0:1024], xbT_d.ap()[it * P : (it + 1) * P, 0:1024]
                )
            nc.sync.dma_start(
                b_bc["bkv"][:, :], bkv_d.ap()[0:1, :].broadcast_to([P, 2 * D])
            )
            for it in range(IT):
                nc.sync.dma_start(
                    xbT_sb[:, it, 1024:2048], xbT_d.ap()[it * P : (it + 1) * P, 1024:2048]
                )
            for dt in range(DT):
                nc.sync.dma_start(
                    bqT[:, dt : dt + 1],
                    bq_d.ap()[0:1, dt * P : (dt + 1) * P].rearrange("a (p w) -> (a p) w", w=1),
                )
            for it in range(IT):
                nc.sync.dma_start(wq_sb[:, it, :], wqT_d.ap()[it * P : (it + 1) * P, :])
            for it in range(IT):
                nc.sync.dma_start(
                    xbT_sb[:, it, 2048:3072], xbT_d.ap()[it * P : (it + 1) * P, 2048:3072]
                )
            nc.sync.dma_start(
                b_bc["bo2"][:, :], bo2_d.ap()[0:1, :].broadcast_to([P, 2 * D])
            )
            for it in range(IT):
                nc.sync.dma_start(
                    xbT_sb[:, it, 3072:4096], xbT_d.ap()[it * P : (it + 1) * P, 3072:4096]
                )
            for it in range(IT):
                nc.sync.dma_start(wo_sb[:, it, :], woT_d.ap()[it * P : (it + 1) * P, :])

            # kv planes: kv_sb[:, 0, t, :] = k (post-softplus),
            #            kv_sb[:, 1, t, :] = v
            kv_sb = big.tile([P, 2, NS, D], FR, tag="kv")
            qT_sb = big.tile([P, DT, SQ], FR, tag="qT")
            GT_sb = wpool.tile([P, DT, D], FR, tag="GT")
            M_sb = wpool.tile([P, DT, D], FR, tag="M")

            # --- kv = x [WkT | WvT] + [bk | bv] ---
            for t in range(NS):
                ts = slice(t * P, (t + 1) * P)
                ps = psKV.tile([P, 2 * D], FP, tag="psKV")
                for it in range(IT):
                    mm(ps[:, :], xbT_sb[:, it, ts], wkv_sb[:, it, :], it == 0, it == IT - 1)
                nc.vector.tensor_tensor(
                    kv_sb[:, :, t, :], ps[:, :].rearrange("p (j d) -> p j d", j=2),
                    b_bc["bkv"][:, :].rearrange("p (j d) -> p j d", j=2), op=ADD,
                )
                # softplus on contiguous k-plane runs of 4 row tiles
                if t % 4 == 3:
                    tt = slice(t - 3, t + 1)
                    tmp = tpool.tile([P, 4, D], FP, tag="tmpk")
                    nc.scalar.activation(tmp[:, :, :], kv_sb[:, 0, tt, :], AF.Exp)
                    nc.scalar.activation(kv_sb[:, 0, tt, :], tmp[:, :, :], AF.Ln, bias=1.0)

            # --- qT = softplus(Wq x^T + bq), transposed layout [e, sq] ---
            for dt in range(DT):
                ds = slice(dt * P, (dt + 1) * P)
                for blk in range(SQ // BLK):
                    ss = slice(blk * BLK, (blk + 1) * BLK)
                    ps = psQ.tile([P, BLK], FP, tag="psQ")
                    for it in range(IT):
                        mm(ps[:, :], wq_sb[:, it, ds], xbT_sb[:, it, ss], it == 0, it == IT - 1)
                    tmp = tpool.tile([P, BLK], FP, tag="tmpq")
                    nc.scalar.activation(tmp[:, :], ps[:, :], AF.Exp, bias=bqT[:, dt : dt + 1])
                    nc.scalar.activation(qT_sb[:, dt, ss], tmp[:, :], AF.Ln, bias=1.0)

            # --- GT[d, e] = sum_s v[s, d] k[s, e] ---
            for dt in range(DT):
                vs = slice(dt * P, (dt + 1) * P)
                ps = psG.tile([P, D], FP, tag="psG")
                for t in range(NS):
                    mm(ps[:, :], kv_sb[:, 1, t, vs], kv_sb[:, 0, t, :], t == 0, t == NS - 1)
                nc.vector.tensor_copy(GT_sb[:, dt, :], ps[:, :])

            # --- M[e, do] = sum_d GT[d, e] WoT[d, do] ---
            for et in range(DT):
                es = slice(et * P, (et + 1) * P)
                ps = psG.tile([P, D], FP, tag="psG")
                for dt in range(DT):
                    mm(ps[:, :], GT_sb[:, dt, es], wo_sb[:, dt, :], dt == 0, dt == DT - 1)
                nc.vector.tensor_copy(M_sb[:, et, :], ps[:, :])

            # --- out[sq, do] = sum_e q[sq, e] M[e, do] + bo, pairs ---
            for pr in range(SQ // (2 * P)):
                ps = psQ.tile([P, 2, D], FP, tag="psQ")
                for j in range(2):
                    ss = slice((2 * pr + j) * P, (2 * pr + j + 1) * P)
                    for et in range(DT):
                        mm(ps[:, j, :], qT_sb[:, et, ss], M_sb[:, et, :], et == 0, et == DT - 1)
                ob = opool.tile([P, 2, D], FP, tag="ob")
                nc.vector.tensor_tensor(
                    ob[:, :, :], ps[:, :, :],
                    b_bc["bo2"][:, :].rearrange("p (j d) -> p j d", j=2), op=ADD,
                )
                nc.sync.dma_start(
                    out_d.ap()[2 * pr * P : (2 * pr + 2) * P, :].rearrange(
                        "(j p) d -> p j d", p=P
                    ),
                    ob[:, :, :],
                )

    # Steer the activation-table pass: only natural_log_exp_and_others
    # (set 6) contains both Exp and Ln, so one PWP table load suffices.
    import concourse.hw_specs as hw_specs

    orig = bacc.get_activation_tables
    bacc.get_activation_tables = _patched_act_tables(hw_specs.get_activation_tables)
    try:
        nc.compile()
    finally:
        bacc.get_activation_tables = orig
    return nc


def _get_nc():
    nc = _CACHE.get("nc")
    if nc is None:
        nc = _build_nc()
        _CACHE["nc"] = nc
    return nc


def make_in_maps(x, Wq, bq, Wk, bk, Wv, bv, Wo, bo):
    B = x.shape[0]
    mmnp = np.float16 if MM_DTYPE_NAME == "float16" else np.float32
    xf = np.asarray(x, dtype=np.float32).reshape(B, S, D)
    xfT = np.ascontiguousarray(xf.transpose(0, 2, 1).astype(mmnp))  # [B, 256, 4096]
    shared = {
        "wqT": np.ascontiguousarray(np.asarray(Wq, mmnp).T),
        "wkvT": np.ascontiguousarray(
            np.hstack([np.asarray(Wk, mmnp).T, np.asarray(Wv, mmnp).T])
        ),
        "woT": np.ascontiguousarray(np.asarray(Wo, mmnp).T),
        "bq": np.asarray(bq, np.float32).reshape(1, D),
        "bkv": np.concatenate(
            [np.asarray(bk, np.float32), np.asarray(bv, np.float32)]
        ).reshape(1, 2 * D),
        "bo2": np.tile(np.asarray(bo, np.float32), 2).reshape(1, 2 * D),
    }
    in_maps = []
    for c in range(N_CORES):
        b, h = divmod(c, 2)
        xT = xfT[b]
        if h == 1:
            xT = np.concatenate([xT[:, SQ:], xT[:, :SQ]], axis=1)
        in_maps.append({"xbT": np.ascontiguousarray(xT), **shared})
    return in_maps


def assemble_out(results, x_shape):
    B, S_, H, W = x_shape
    out = np.empty((B, S_, D), np.float32)
    for c in range(N_CORES):
        b, h = divmod(c, 2)
        out[b, h * SQ : (h + 1) * SQ] = results[c]["out"]
    return out.reshape(B, S_, H, W)


def kernel(x, Wq, bq, Wk, bk, Wv, bv, Wo, bo, _trace=False):
    from concourse.bass_utils import run_bass_kernel_spmd

    nc = _get_nc()
    in_maps = make_in_maps(x, Wq, bq, Wk, bk, Wv, bv, Wo, bo)
    res = run_bass_kernel_spmd(nc, in_maps, list(range(N_CORES)), trace=_trace)
    out = assemble_out(res.results, x.shape)
    if _trace:
        _CACHE["last_result"] = res
    return out
